# revision 3
# baseline (speedup 1.0000x reference)
"""CPCC loss (1 - Pearson(tree_d, proto_d)) on 8 Trainium2 NeuronCores.

Strategy (data-parallel, per sharding hint):
  - Shard representations/target_fine along N across the 8 cores (contiguous
    32768-row blocks).
  - Each core streams its 16 MiB of representations from HBM; for every
    128-row chunk it builds a one-hot [128, C_FINE] on DVE (is_equal against
    an iota constant) and accumulates  onehot.T @ [reps | 1]  into a single
    [C_FINE, D+1] PSUM tile -> per-core segment sums + counts.
  - AllGather the [100, 129] partials (cheaper than AllReduce at this size),
    tree-sum locally, then every core runs the tiny replicated tail:
    prototypes -> pairwise distance matrices via Gram trick -> tree-distance
    expansion -> masked Pearson correlation -> scalar loss.
"""

import numpy as np

C_FINE, C_MID, C_COARSE = 100, 20, 5
EPS = 1e-12
N_CORES = 8
N, D = 262144, 128
N_LOC = N // N_CORES            # 32768 rows per core
CHUNK = 128                     # contraction size per matmul
N_CHUNKS = N_LOC // CHUNK       # 256
TILE_CHUNKS = 32                # chunks per DMA tile (4096 rows = 2 MiB)
N_TILES = N_CHUNKS // TILE_CHUNKS
NPAIRS = C_FINE * (C_FINE - 1) // 2   # 4950

_CACHE = {}


def _build_program():
    import concourse.bacc as bacc
    import concourse.mybir as mybir
    import concourse.tile as tile
    from concourse.bass import MemorySpace

    f32 = mybir.dt.float32
    i32 = mybir.dt.int32
    Alu = mybir.AluOpType
    Act = mybir.ActivationFunctionType

    nc = bacc.Bacc("TRN2", target_bir_lowering=False, debug=False,
                   num_devices=N_CORES)

    reps_d = nc.dram_tensor("reps", [N_LOC, D], f32, kind="ExternalInput")
    tgtT_d = nc.dram_tensor("tgtT", [CHUNK, N_CHUNKS], i32, kind="ExternalInput")
    iota_d = nc.dram_tensor("iota", [128, C_FINE], f32, kind="ExternalInput")
    ident_d = nc.dram_tensor("ident", [128, 128], f32, kind="ExternalInput")
    ones_d = nc.dram_tensor("ones", [128, 128], f32, kind="ExternalInput")
    emt_d = nc.dram_tensor("emt", [C_MID, C_FINE], f32, kind="ExternalInput")
    ect_d = nc.dram_tensor("ect", [C_COARSE, C_FINE], f32, kind="ExternalInput")
    wm_d = nc.dram_tensor("wm", [C_FINE, C_MID], f32, kind="ExternalInput")
    wc_d = nc.dram_tensor("wc", [C_MID, C_COARSE], f32, kind="ExternalInput")
    mask_d = nc.dram_tensor("mask", [C_FINE, C_FINE], f32, kind="ExternalInput")
    loss_d = nc.dram_tensor("loss", [1, 1], f32, kind="ExternalOutput")

    with tile.TileContext(nc) as tc:
        with (
            tc.tile_pool(name="const", bufs=1) as cpool,
            tc.tile_pool(name="reps", bufs=4) as rpool,
            tc.tile_pool(name="oh", bufs=8) as opool,
            tc.tile_pool(name="work", bufs=1) as wpool,
            tc.tile_pool(name="acc", bufs=1, space=MemorySpace.PSUM) as apool,
            tc.tile_pool(name="tps", bufs=4, space=MemorySpace.PSUM) as ppool,
            tc.tile_pool(name="dram", bufs=1, space=MemorySpace.DRAM) as dpool,
        ):
            # ---- constants ----
            iota_t = cpool.tile([128, C_FINE], f32)
            nc.sync.dma_start(iota_t[:], iota_d[:])
            ident_t = cpool.tile([128, 128], f32)
            nc.sync.dma_start(ident_t[:], ident_d[:])
            ones_t = cpool.tile([128, 128], f32)
            nc.sync.dma_start(ones_t[:], ones_d[:])
            emt_t = cpool.tile([C_MID, C_FINE], f32)
            nc.sync.dma_start(emt_t[:], emt_d[:])
            ect_t = cpool.tile([C_COARSE, C_FINE], f32)
            nc.sync.dma_start(ect_t[:], ect_d[:])
            wm_t = cpool.tile([C_FINE, C_MID], f32)
            nc.sync.dma_start(wm_t[:], wm_d[:])
            wc_t = cpool.tile([C_MID, C_COARSE], f32)
            nc.sync.dma_start(wc_t[:], wc_d[:])
            mask_t = cpool.tile([C_FINE, C_FINE], f32)
            nc.sync.dma_start(mask_t[:], mask_d[:])

            tgti_t = cpool.tile([CHUNK, N_CHUNKS], i32)
            nc.sync.dma_start(tgti_t[:], tgtT_d[:])
            tgtf_t = cpool.tile([CHUNK, N_CHUNKS], f32)
            nc.vector.tensor_copy(tgtf_t[:], tgti_t[:])

            eps_t = cpool.tile([128, 1], f32)
            nc.vector.memset(eps_t[:], EPS)

            # ---- main streaming loop: segment sums + counts ----
            acc = apool.tile([C_FINE, D + 1], f32)
            for t in range(N_TILES):
                rt = rpool.tile([128, TILE_CHUNKS, D + 1], f32, tag="rt")
                src = reps_d[t * TILE_CHUNKS * CHUNK:(t + 1) * TILE_CHUNKS * CHUNK, :]
                nc.sync.dma_start(
                    rt[:, :, 0:D], src.rearrange("(k p) d -> p k d", p=128)
                )
                nc.gpsimd.memset(rt[:, :, D:D + 1], 1.0)
                for k in range(TILE_CHUNKS):
                    c = t * TILE_CHUNKS + k
                    oh = opool.tile([128, C_FINE], f32, tag="oh")
                    nc.vector.tensor_scalar(
                        oh[:], iota_t[:], tgtf_t[:, c:c + 1], None, Alu.is_equal
                    )
                    nc.tensor.matmul(
                        acc[:], oh[:], rt[:, k, :],
                        start=(c == 0), stop=(c == N_CHUNKS - 1),
                    )

            part_t = wpool.tile([C_FINE, D + 1], f32)
            nc.vector.tensor_copy(part_t[:], acc[:])

            # ---- cross-core AllGather + local sum ----
            cc_in = dpool.tile([C_FINE, D + 1], f32)
            cc_out = dpool.tile([N_CORES * C_FINE, D + 1], f32)
            nc.sync.dma_start(cc_in[:], part_t[:])
            nc.gpsimd.collective_compute(
                "AllGather",
                mybir.AluOpType.bypass,
                replica_groups=[list(range(N_CORES))],
                ins=[cc_in.opt()],
                outs=[cc_out.opt()],
            )
            gath = wpool.tile([C_FINE, N_CORES, D + 1], f32)
            nc.sync.dma_start(
                gath[:], cc_out[:].rearrange("(r c) f -> c r f", r=N_CORES)
            )
            nc.vector.tensor_add(gath[:, 0:4, :], gath[:, 0:4, :], gath[:, 4:8, :])
            nc.vector.tensor_add(gath[:, 0:2, :], gath[:, 0:2, :], gath[:, 2:4, :])
            S = wpool.tile([C_FINE, D + 1], f32)
            nc.vector.tensor_add(S[:], gath[:, 0, :], gath[:, 1, :])

            # ---- fine prototypes: sums / max(count, 1) ----
            cnt_m = wpool.tile([C_FINE, 1], f32)
            nc.vector.tensor_scalar_max(cnt_m[:], S[:, D:D + 1], 1.0)
            rec = wpool.tile([C_FINE, 1], f32)
            nc.vector.reciprocal(rec[:], cnt_m[:])
            P_t = wpool.tile([C_FINE, D], f32)
            nc.vector.tensor_scalar_mul(P_t[:], S[:, 0:D], rec[:])

            # ---- mid / coarse prototypes ----
            ps_m = ppool.tile([C_MID, D], f32, tag="tps")
            nc.tensor.matmul(ps_m[:], wm_t[:], P_t[:], start=True, stop=True)
            M_t = wpool.tile([C_MID, D], f32)
            nc.vector.tensor_copy(M_t[:], ps_m[:])
            ps_c = ppool.tile([C_COARSE, D], f32, tag="tps")
            nc.tensor.matmul(ps_c[:], wc_t[:], M_t[:], start=True, stop=True)
            C_t = wpool.tile([C_COARSE, D], f32)
            nc.vector.tensor_copy(C_t[:], ps_c[:])

            # ---- transposes: [n, D] -> [D, n] ----
            def transpose_to_sbuf(src_t, n):
                ps = ppool.tile([D, n], f32, tag="tps")
                nc.tensor.transpose(ps[:], src_t[:], ident_t[0:n, 0:n])
                sb = wpool.tile([D, n], f32, tag=f"tr{n}")
                nc.vector.tensor_copy(sb[:], ps[:])
                return sb

            PT_s = transpose_to_sbuf(P_t, C_FINE)
            MT_s = transpose_to_sbuf(M_t, C_MID)
            CT_s = transpose_to_sbuf(C_t, C_COARSE)

            # ---- pairwise distance matrix via Gram trick ----
            # d2[i,j] = n_i + n_j - 2 G[i,j];  psum = G - (n_i + n_j)/2
            # dist = sqrt(max(-2*psum, 0) + EPS)
            def dist_matrix(XT_s, n):
                x2 = wpool.tile([D, n], f32, tag=f"x2{n}")
                nc.vector.tensor_mul(x2[:], XT_s[:], XT_s[:])
                ps_n = ppool.tile([1, n], f32, tag="tps")
                nc.tensor.matmul(ps_n[:], ones_t[:, 0:1], x2[:],
                                 start=True, stop=True)
                nm = wpool.tile([1, n], f32, tag=f"nm{n}")
                nc.vector.tensor_scalar(nm[:], ps_n[:], -0.5, None, Alu.mult)
                ps_g = ppool.tile([n, n], f32, tag="tps")
                nc.tensor.matmul(ps_g[:], XT_s[:], XT_s[:], start=True, stop=False)
                nc.tensor.matmul(ps_g[:], ones_t[0:1, 0:n], nm[:],
                                 start=False, stop=False)
                nc.tensor.matmul(ps_g[:], nm[:], ones_t[0:1, 0:n],
                                 start=False, stop=True)
                d2 = wpool.tile([n, n], f32, tag=f"d2{n}")
                nc.vector.tensor_scalar(d2[:], ps_g[:], -2.0, 0.0,
                                        Alu.mult, Alu.max)
                dist = wpool.tile([n, n], f32, tag=f"dm{n}")
                nc.scalar.activation(dist[:], d2[:], Act.Sqrt,
                                     bias=eps_t[0:n, 0:1], scale=1.0)
                return dist

            Df_s = dist_matrix(PT_s, C_FINE)     # fine proto distances
            Dm_s = dist_matrix(MT_s, C_MID)      # mid proto distances
            Dc_s = dist_matrix(CT_s, C_COARSE)   # coarse proto distances

            # ---- expand to tree distance matrix T[i,j] over fine pairs ----
            ps_ym = ppool.tile([C_MID, C_FINE], f32, tag="tps")
            nc.tensor.matmul(ps_ym[:], Dm_s[:], emt_t[:], start=True, stop=True)
            Ym_s = wpool.tile([C_MID, C_FINE], f32)
            nc.vector.tensor_copy(Ym_s[:], ps_ym[:])
            ps_yc = ppool.tile([C_COARSE, C_FINE], f32, tag="tps")
            nc.tensor.matmul(ps_yc[:], Dc_s[:], ect_t[:], start=True, stop=True)
            Yc_s = wpool.tile([C_COARSE, C_FINE], f32)
            nc.vector.tensor_copy(Yc_s[:], ps_yc[:])
            ps_T = ppool.tile([C_FINE, C_FINE], f32, tag="tps")
            nc.tensor.matmul(ps_T[:], emt_t[:], Ym_s[:], start=True, stop=False)
            nc.tensor.matmul(ps_T[:], ect_t[:], Yc_s[:], start=False, stop=True)

            # ---- masked sums for Pearson correlation ----
            # F1=sum(T*mask) F2=sum(P*mask) F3=sum(T*P*mask)
            # F4=sum(T*T*mask) F5=sum(P*P*mask)   (full-matrix = 2x pair sums)
            red = wpool.tile([C_FINE, 8], f32)
            Tm_s = wpool.tile([C_FINE, C_FINE], f32)
            nc.vector.scalar_tensor_tensor(
                Tm_s[:], ps_T[:], 1.0, mask_t[:], Alu.mult, Alu.mult,
                accum_out=red[:, 0:1])
            Pm_s = wpool.tile([C_FINE, C_FINE], f32)
            nc.vector.scalar_tensor_tensor(
                Pm_s[:], Df_s[:], 1.0, mask_t[:], Alu.mult, Alu.mult,
                accum_out=red[:, 1:2])
            tp_s = wpool.tile([C_FINE, C_FINE], f32)
            nc.vector.scalar_tensor_tensor(
                tp_s[:], Tm_s[:], 1.0, Df_s[:], Alu.mult, Alu.mult,
                accum_out=red[:, 2:3])
            tt_s = wpool.tile([C_FINE, C_FINE], f32)
            nc.vector.scalar_tensor_tensor(
                tt_s[:], Tm_s[:], 1.0, ps_T[:], Alu.mult, Alu.mult,
                accum_out=red[:, 3:4])
            pp_s = wpool.tile([C_FINE, C_FINE], f32)
            nc.vector.scalar_tensor_tensor(
                pp_s[:], Pm_s[:], 1.0, Df_s[:], Alu.mult, Alu.mult,
                accum_out=red[:, 4:5])

            ps_red = ppool.tile([1, 5], f32, tag="tps")
            nc.tensor.matmul(ps_red[:], ones_t[0:C_FINE, 0:1], red[:, 0:5],
                             start=True, stop=True)
            f_s = wpool.tile([1, 5], f32)
            nc.vector.tensor_copy(f_s[:], ps_red[:])

            # ---- final scalars ----
            # num = F3/2 - F1*F2/19800 ; dt = F4/2 - F1^2/19800
            # dp = F5/2 - F2^2/19800 ; loss = 1 - num/sqrt(dt*dp + EPS)
            inv = 1.0 / (4.0 * NPAIRS)
            g1 = wpool.tile([1, 1], f32)
            nc.vector.tensor_scalar(g1[:], f_s[:, 0:1], inv, None, Alu.mult)
            g2 = wpool.tile([1, 1], f32)
            nc.vector.tensor_scalar(g2[:], f_s[:, 1:2], inv, None, Alu.mult)
            t_ab = wpool.tile([1, 1], f32)
            nc.vector.tensor_mul(t_ab[:], g1[:], f_s[:, 1:2])
            t_aa = wpool.tile([1, 1], f32)
            nc.vector.tensor_mul(t_aa[:], g1[:], f_s[:, 0:1])
            t_bb = wpool.tile([1, 1], f32)
            nc.vector.tensor_mul(t_bb[:], g2[:], f_s[:, 1:2])
            num = wpool.tile([1, 1], f32)
            nc.vector.scalar_tensor_tensor(
                num[:], f_s[:, 2:3], 0.5, t_ab[:], Alu.mult, Alu.subtract)
            dt = wpool.tile([1, 1], f32)
            nc.vector.scalar_tensor_tensor(
                dt[:], f_s[:, 3:4], 0.5, t_aa[:], Alu.mult, Alu.subtract)
            dp = wpool.tile([1, 1], f32)
            nc.vector.scalar_tensor_tensor(
                dp[:], f_s[:, 4:5], 0.5, t_bb[:], Alu.mult, Alu.subtract)
            den = wpool.tile([1, 1], f32)
            nc.vector.scalar_tensor_tensor(
                den[:], dt[:], 1.0, dp[:], Alu.mult, Alu.mult)
            dene = wpool.tile([1, 1], f32)
            nc.vector.tensor_scalar(dene[:], den[:], EPS, None, Alu.add)
            sq = wpool.tile([1, 1], f32)
            nc.scalar.activation(sq[:], dene[:], Act.Sqrt)
            rsq = wpool.tile([1, 1], f32)
            nc.vector.reciprocal(rsq[:], sq[:])
            corr = wpool.tile([1, 1], f32)
            nc.vector.tensor_mul(corr[:], num[:], rsq[:])
            loss_t = wpool.tile([1, 1], f32)
            nc.vector.tensor_scalar(loss_t[:], corr[:], -1.0, 1.0,
                                    Alu.mult, Alu.add)
            nc.sync.dma_start(loss_d[:], loss_t[:])

    nc.compile()
    return nc


def _host_constants(fine2mid, fine2coarse):
    f2m = np.asarray(fine2mid, dtype=np.int64)
    f2c = np.asarray(fine2coarse, dtype=np.int64)
    iota = np.broadcast_to(
        np.arange(C_FINE, dtype=np.float32), (128, C_FINE)).copy()
    ident = np.eye(128, dtype=np.float32)
    ones = np.ones((128, 128), dtype=np.float32)
    emt = (f2m[None, :] == np.arange(C_MID)[:, None]).astype(np.float32)
    ect_sel = np.zeros((C_COARSE, C_FINE), dtype=np.float32)
    # tree coarse distance is looked up through mid2coarse = segment_max
    cnt_m = np.maximum(np.bincount(f2m, minlength=C_MID), 1).astype(np.float32)
    wm = (emt / cnt_m[:, None]).T.astype(np.float32).copy()   # [C_FINE, C_MID]
    # mid2coarse[m] = segment_max of fine2coarse over fines with fine2mid==m
    m2c = np.full(C_MID, -(2**31), dtype=np.int64)
    np.maximum.at(m2c, f2m, f2c)
    emc = (m2c[None, :] == np.arange(C_COARSE)[:, None]).astype(np.float32)
    cnt_c = np.maximum(np.bincount(np.clip(m2c, 0, C_COARSE - 1),
                                   weights=np.ones(C_MID),
                                   minlength=C_COARSE), 1).astype(np.float32)
    # counts for coarse = number of mids mapped to each coarse
    cnt_c = np.maximum(emc.sum(axis=1), 1).astype(np.float32)
    wc = (emc / cnt_c[:, None]).T.astype(np.float32).copy()   # [C_MID, C_COARSE]
    # expansion selectors: ect[c, f] = (fine2coarse[f] == c)
    ect_sel = (f2c[None, :] == np.arange(C_COARSE)[:, None]).astype(np.float32)
    mask = (1.0 - np.eye(C_FINE)).astype(np.float32)
    return {
        "iota": iota, "ident": ident, "ones": ones,
        "emt": np.ascontiguousarray(emt),
        "ect": np.ascontiguousarray(ect_sel),
        "wm": np.ascontiguousarray(wm),
        "wc": np.ascontiguousarray(wc),
        "mask": mask,
    }


def kernel(representations, target_fine, fine2mid, fine2coarse):
    from concourse.bass_utils import run_bass_kernel_spmd

    reps = np.ascontiguousarray(np.asarray(representations, dtype=np.float32))
    tgt = np.asarray(target_fine, dtype=np.int32)
    assert reps.shape == (N, D) and tgt.shape == (N,)

    if "nc" not in _CACHE:
        _CACHE["nc"] = _build_program()
    nc = _CACHE["nc"]

    consts = _host_constants(fine2mid, fine2coarse)
    in_maps = []
    for r in range(N_CORES):
        lo, hi = r * N_LOC, (r + 1) * N_LOC
        tgtT = np.ascontiguousarray(
            tgt[lo:hi].reshape(N_CHUNKS, CHUNK).T)      # [128, 256] int32
        in_maps.append({
            "reps": reps[lo:hi],
            "tgtT": tgtT,
            **consts,
        })

    res = run_bass_kernel_spmd(nc, in_maps, core_ids=list(range(N_CORES)))
    loss = res.results[0]["loss"][0, 0]
    return np.asarray(loss, dtype=np.float32).reshape(())


# revision 41
# speedup vs baseline: 37352.3639x; 37352.3639x over previous
"""CPCC loss (1 - Pearson(tree_d, proto_d)) on 8 Trainium2 NeuronCores.

Strategy (data-parallel, per sharding hint):
  - Shard representations/target_fine along N across the 8 cores (contiguous
    32768-row blocks).
  - Each core streams its 16 MiB of representations from HBM (SWDGE DMA with
    inline f32->bf16 cast); for every 128-row chunk a bf16 one-hot
    [128 tokens x 128 classes(padded)] is built on DVE (is_equal against an
    iota constant, 16 chunks per instruction via a broadcast AP) and one PE
    matmul per chunk accumulates  onehot.T @ [reps | 1]  into a [128, 129]
    f32 PSUM tile -> per-core segment sums + counts. The 128-wide (padded)
    one-hot keeps LDWEIGHTS on the fast-weight-load path; pad columns never
    match so their psum rows stay zero.
  - AllGather the [100, 129] partials (cheaper than AllReduce at this size),
    tree-sum locally, then every core runs the tiny replicated tail:
    prototypes -> pairwise distance matrices via the Gram trick (clamped at
    zero before sqrt) -> tree-distance expansion -> Pearson correlation ->
    scalar loss. Off-diagonal masking is skipped: diagonal terms are
    O(sqrt(EPS)) = 1e-6, negligible against sums of ~1e3 in f32.

Precision: only the representations are rounded to bf16 (matmul operand);
accumulation is f32 in PSUM and the whole tail is f32. Observed loss error
vs the f32 reference is ~1e-4 relative.
"""

import numpy as np

C_FINE, C_MID, C_COARSE = 100, 20, 5
EPS = 1e-12
N_CORES = 8
N, D = 262144, 128
N_LOC = N // N_CORES            # 32768 rows per core
CHUNK = 128                     # contraction size per matmul
N_CHUNKS = N_LOC // CHUNK       # 256
TILE_CHUNKS = 32                # chunks per DMA tile (4096 rows = 2 MiB)
N_TILES = N_CHUNKS // TILE_CHUNKS
OH_BATCH = 16                   # one-hot chunks built per DVE op
NPAIRS = C_FINE * (C_FINE - 1) // 2   # 4950

_CACHE = {}


def _build_program(stream_reps=1, loop_reps=1, dma_only=False, no_cc=False,
                   cc_tail_reps=1):
    """Build the SPMD program.

    Benchmarking knobs (the graded kernel uses all defaults):
      stream_reps>1 statically unrolls the streaming phase (same data).
      loop_reps>1 wraps the streaming phase in a dynamic For_i loop (slope
        timing); psum restarts each rep so the output stays correct.
      dma_only=True keeps only 1 matmul/one-hot batch per tile.
      no_cc=True builds a single-core program without the AllGather (for
        TimelineSim cost-model analysis).
      cc_tail_reps>1 serially chains the AllGather+tail section that many
        times (slope timing of the non-streaming part; output garbage).
    """
    import contextlib

    import concourse.bacc as bacc
    import concourse.mybir as mybir
    import concourse.tile as tile
    from concourse.bass import MemorySpace
    from concourse.tile import add_dep_helper

    f32 = mybir.dt.float32
    bf16 = mybir.dt.bfloat16
    i32 = mybir.dt.int32
    Alu = mybir.AluOpType
    Act = mybir.ActivationFunctionType
    X = mybir.AxisListType.X

    nc = bacc.Bacc("TRN2", target_bir_lowering=False, debug=False,
                   num_devices=1 if no_cc else N_CORES)

    reps_d = nc.dram_tensor("reps", [N_LOC, D], f32, kind="ExternalInput")
    tgtT_d = nc.dram_tensor("tgtT", [CHUNK, N_CHUNKS], i32, kind="ExternalInput")
    iota_d = nc.dram_tensor("iota", [128, OH_BATCH * CHUNK], bf16,
                            kind="ExternalInput")
    ident_d = nc.dram_tensor("ident", [128, 128], f32, kind="ExternalInput")
    ones_d = nc.dram_tensor("ones", [128, 128], f32, kind="ExternalInput")
    emt_d = nc.dram_tensor("emt", [C_MID, C_FINE], f32, kind="ExternalInput")
    ect_d = nc.dram_tensor("ect", [C_COARSE, C_FINE], f32, kind="ExternalInput")
    wm_d = nc.dram_tensor("wm", [C_FINE, C_MID], f32, kind="ExternalInput")
    wc_d = nc.dram_tensor("wc", [C_MID, C_COARSE], f32, kind="ExternalInput")
    loss_d = nc.dram_tensor("loss", [1, 1], f32, kind="ExternalOutput")

    with tile.TileContext(nc) as tc:
        with (
            tc.tile_pool(name="const", bufs=1) as cpool,
            tc.tile_pool(name="reps", bufs=4) as rpool,
            tc.tile_pool(name="oh", bufs=3) as opool,
            tc.tile_pool(name="work", bufs=1) as wpool,
            tc.tile_pool(name="acc", bufs=1, space=MemorySpace.PSUM) as apool,
            tc.tile_pool(name="tps", bufs=4, space=MemorySpace.PSUM) as ppool,
            tc.tile_pool(name="dram", bufs=1, space=MemorySpace.DRAM) as dpool,
        ):
            # ---- constants (target first: it gates the whole DVE chain) ----
            tgti_t = cpool.tile([CHUNK, N_CHUNKS], i32)
            nc.sync.dma_start(tgti_t[:], tgtT_d[:])
            tgtf_t = cpool.tile([CHUNK, N_CHUNKS], bf16)
            nc.vector.tensor_copy(tgtf_t[:], tgti_t[:])
            iota_t = cpool.tile([128, OH_BATCH, CHUNK], bf16)
            nc.sync.dma_start(
                iota_t[:],
                iota_d[:].rearrange("p (g c) -> p g c", c=CHUNK))

            ident_t = cpool.tile([128, 128], f32)
            nc.sync.dma_start(ident_t[:], ident_d[:])
            ones_t = cpool.tile([128, 128], f32)
            nc.sync.dma_start(ones_t[:], ones_d[:])
            emt_t = cpool.tile([C_MID, C_FINE], f32)
            nc.sync.dma_start(emt_t[:], emt_d[:])
            ect_t = cpool.tile([C_COARSE, C_FINE], f32)
            nc.sync.dma_start(ect_t[:], ect_d[:])
            wm_t = cpool.tile([C_FINE, C_MID], f32)
            nc.sync.dma_start(wm_t[:], wm_d[:])
            wc_t = cpool.tile([C_MID, C_COARSE], f32)
            nc.sync.dma_start(wc_t[:], wc_d[:])
            eps_t = cpool.tile([128, 1], f32)
            nc.vector.memset(eps_t[:], EPS)

            # ---- main streaming loop: segment sums + counts ----
            acc = apool.tile([CHUNK, D + 1], f32)
            loop_cm = (tc.For_i(0, loop_reps, 1) if loop_reps > 1
                       else contextlib.nullcontext())
            with loop_cm:
                for rep in range(stream_reps):
                    for t in range(N_TILES):
                        rt = rpool.tile([128, TILE_CHUNKS, D + 1], bf16,
                                        tag="rt")
                        src = reps_d[t * TILE_CHUNKS * CHUNK:
                                     (t + 1) * TILE_CHUNKS * CHUNK, :]
                        # ones column for the counts. Must complete BEFORE the
                        # cast-DMA: the DMA's 256B bf16 runs read-modify-write
                        # the surrounding SBUF lines, so a concurrent fill of
                        # the interleaved ones bytes would be lost (this race
                        # was observed as a +-1e-4 run-to-run loss jitter).
                        ms = nc.vector.memset(rt[:, :, D], 1.0)
                        # row = p*TILE_CHUNKS + k -> each partition reads one
                        # fully contiguous block from HBM; SWDGE casts to bf16
                        dm = nc.gpsimd.dma_start(
                            rt[:, :, 0:D],
                            src.rearrange("(p k) d -> p k d", k=TILE_CHUNKS)
                        )
                        add_dep_helper(dm.ins, ms.ins, sync=True,
                                       reason="ones col before cast-DMA RMW")
                        n_b = 1 if dma_only else TILE_CHUNKS // OH_BATCH
                        for b in range(n_b):
                            oh = opool.tile([128, OH_BATCH, CHUNK], bf16,
                                            tag="oh")
                            c0 = t * TILE_CHUNKS + b * OH_BATCH
                            tgt_b = (tgtf_t[:, c0:c0 + OH_BATCH]
                                     .rearrange("p (g o) -> p g o", o=1)
                                     .broadcast_to([128, OH_BATCH, CHUNK]))
                            nc.vector.tensor_tensor(
                                oh[:], iota_t[:], tgt_b, Alu.is_equal)
                            js = [0] if dma_only else range(OH_BATCH)
                            for j in js:
                                k = b * OH_BATCH + j
                                nc.tensor.matmul(
                                    acc[:], oh[:, j, :], rt[:, k, :],
                                    start=(rep == 0 and t == 0 and k == 0),
                                    stop=(rep == stream_reps - 1
                                          and t == N_TILES - 1
                                          and (k == TILE_CHUNKS - 1
                                               or dma_only)),
                                )

            part_t = wpool.tile([C_FINE, D + 1], f32)
            nc.vector.tensor_copy(part_t[:], acc[0:C_FINE, :])

            if not no_cc:
                cc_in = dpool.tile([C_FINE, D + 1], f32)
                cc_out = dpool.tile([N_CORES * C_FINE, D + 1], f32)

            def gather_summed():
                """AllGather the per-core partials and sum them locally."""
                if no_cc:
                    return part_t
                nc.sync.dma_start(cc_in[:], part_t[:])
                nc.gpsimd.collective_compute(
                    "AllGather",
                    mybir.AluOpType.bypass,
                    replica_groups=[list(range(N_CORES))],
                    ins=[cc_in.opt()],
                    outs=[cc_out.opt()],
                )
                gath = wpool.tile([C_FINE, N_CORES, D + 1], f32)
                nc.sync.dma_start(
                    gath[:],
                    cc_out[:].rearrange("(r c) f -> c r f", r=N_CORES))
                nc.vector.tensor_add(gath[:, 0:4, :], gath[:, 0:4, :],
                                     gath[:, 4:8, :])
                nc.vector.tensor_add(gath[:, 0:2, :], gath[:, 0:2, :],
                                     gath[:, 2:4, :])
                S = wpool.tile([C_FINE, D + 1], f32)
                nc.vector.tensor_add(S[:], gath[:, 0, :], gath[:, 1, :])
                return S

            def transpose_to_sbuf(src_t, n):
                ps = ppool.tile([D, n], f32, tag="tps")
                nc.tensor.transpose(ps[:], src_t[:], ident_t[0:n, 0:n])
                sb = wpool.tile([D, n], f32, tag=f"tr{n}")
                nc.vector.tensor_copy(sb[:], ps[:])
                return sb

            def dist_matrix(XT_s, n):
                # d2[i,j] = n_i + n_j - 2 G[i,j];  psum = G - (n_i + n_j)/2
                # dist = sqrt(max(-2*psum, 0) + EPS)
                x2 = wpool.tile([D, n], f32, tag=f"x2{n}")
                nc.vector.tensor_mul(x2[:], XT_s[:], XT_s[:])
                ps_n = ppool.tile([1, n], f32, tag="tps")
                nc.tensor.matmul(ps_n[:], ones_t[:, 0:1], x2[:],
                                 start=True, stop=True)
                nm = wpool.tile([1, n], f32, tag=f"nm{n}")
                nc.vector.tensor_scalar(nm[:], ps_n[:], -0.5, None, Alu.mult)
                ps_g = ppool.tile([n, n], f32, tag="tps")
                nc.tensor.matmul(ps_g[:], XT_s[:], XT_s[:],
                                 start=True, stop=False)
                nc.tensor.matmul(ps_g[:], ones_t[0:1, 0:n], nm[:],
                                 start=False, stop=False)
                nc.tensor.matmul(ps_g[:], nm[:], ones_t[0:1, 0:n],
                                 start=False, stop=True)
                d2 = wpool.tile([n, n], f32, tag=f"d2{n}")
                nc.vector.tensor_scalar(d2[:], ps_g[:], -2.0, 0.0,
                                        Alu.mult, Alu.max)
                dist = wpool.tile([n, n], f32, tag=f"dm{n}")
                nc.scalar.activation(dist[:], d2[:], Act.Sqrt,
                                     bias=eps_t[0:n, 0:1], scale=1.0)
                return dist

            def tail(S):
                # fine prototypes: sums / max(count, 1)
                cnt_m = wpool.tile([C_FINE, 1], f32)
                nc.vector.tensor_scalar_max(cnt_m[:], S[:, D:D + 1], 1.0)
                rec = wpool.tile([C_FINE, 1], f32)
                nc.vector.reciprocal(rec[:], cnt_m[:])
                P_t = wpool.tile([C_FINE, D], f32)
                nc.vector.tensor_scalar_mul(P_t[:], S[:, 0:D], rec[:])

                # mid / coarse prototypes
                ps_m = ppool.tile([C_MID, D], f32, tag="tps")
                nc.tensor.matmul(ps_m[:], wm_t[:], P_t[:],
                                 start=True, stop=True)
                M_t = wpool.tile([C_MID, D], f32)
                nc.vector.tensor_copy(M_t[:], ps_m[:])
                ps_c = ppool.tile([C_COARSE, D], f32, tag="tps")
                nc.tensor.matmul(ps_c[:], wc_t[:], M_t[:],
                                 start=True, stop=True)
                C_t = wpool.tile([C_COARSE, D], f32)
                nc.vector.tensor_copy(C_t[:], ps_c[:])

                PT_s = transpose_to_sbuf(P_t, C_FINE)
                MT_s = transpose_to_sbuf(M_t, C_MID)
                CT_s = transpose_to_sbuf(C_t, C_COARSE)

                Df_s = dist_matrix(PT_s, C_FINE)    # fine proto distances
                Dm_s = dist_matrix(MT_s, C_MID)     # mid proto distances
                Dc_s = dist_matrix(CT_s, C_COARSE)  # coarse proto distances

                # expand to tree distance matrix T[i,j] over fine pairs
                ps_ym = ppool.tile([C_MID, C_FINE], f32, tag="tps")
                nc.tensor.matmul(ps_ym[:], Dm_s[:], emt_t[:],
                                 start=True, stop=True)
                Ym_s = wpool.tile([C_MID, C_FINE], f32)
                nc.vector.tensor_copy(Ym_s[:], ps_ym[:])
                ps_yc = ppool.tile([C_COARSE, C_FINE], f32, tag="tps")
                nc.tensor.matmul(ps_yc[:], Dc_s[:], ect_t[:],
                                 start=True, stop=True)
                Yc_s = wpool.tile([C_COARSE, C_FINE], f32)
                nc.vector.tensor_copy(Yc_s[:], ps_yc[:])
                ps_T = ppool.tile([C_FINE, C_FINE], f32, tag="tps")
                nc.tensor.matmul(ps_T[:], emt_t[:], Ym_s[:],
                                 start=True, stop=False)
                nc.tensor.matmul(ps_T[:], ect_t[:], Yc_s[:],
                                 start=False, stop=True)

                # Pearson sums. Diagonal contributions are O(sqrt(EPS))=1e-6
                # vs sums ~1e3 — negligible in f32, so no masking needed.
                # F1=sum(T) F2=sum(P) F3=sum(T*P) F4=sum(T^2) F5=sum(P^2)
                red = wpool.tile([C_FINE, 8], f32)
                Tsb = wpool.tile([C_FINE, C_FINE], f32)
                nc.vector.tensor_scalar(
                    Tsb[:], ps_T[:], 1.0, 0.0, Alu.mult, Alu.add,
                    accum_out=red[:, 0:1])
                nc.vector.reduce_sum(red[:, 1:2], Df_s[:], axis=X)
                tp_s = wpool.tile([C_FINE, C_FINE], f32)
                nc.vector.scalar_tensor_tensor(
                    tp_s[:], Tsb[:], 1.0, Df_s[:], Alu.mult, Alu.mult,
                    accum_out=red[:, 2:3])
                tt_s = wpool.tile([C_FINE, C_FINE], f32)
                nc.vector.scalar_tensor_tensor(
                    tt_s[:], Tsb[:], 1.0, Tsb[:], Alu.mult, Alu.mult,
                    accum_out=red[:, 3:4])
                pp_s = wpool.tile([C_FINE, C_FINE], f32)
                nc.vector.scalar_tensor_tensor(
                    pp_s[:], Df_s[:], 1.0, Df_s[:], Alu.mult, Alu.mult,
                    accum_out=red[:, 4:5])

                ps_red = ppool.tile([1, 5], f32, tag="tps")
                nc.tensor.matmul(ps_red[:], ones_t[0:C_FINE, 0:1],
                                 red[:, 0:5], start=True, stop=True)
                f_s = wpool.tile([1, 5], f32)
                nc.vector.tensor_copy(f_s[:], ps_red[:])

                # num = F3/2 - F1*F2/19800 ; dt = F4/2 - F1^2/19800
                # dp = F5/2 - F2^2/19800 ; loss = 1 - num/sqrt(dt*dp + EPS)
                inv = 1.0 / (4.0 * NPAIRS)
                g1 = wpool.tile([1, 1], f32)
                nc.vector.tensor_scalar(g1[:], f_s[:, 0:1], inv, None,
                                        Alu.mult)
                g2 = wpool.tile([1, 1], f32)
                nc.vector.tensor_scalar(g2[:], f_s[:, 1:2], inv, None,
                                        Alu.mult)
                t_ab = wpool.tile([1, 1], f32)
                nc.vector.tensor_mul(t_ab[:], g1[:], f_s[:, 1:2])
                t_aa = wpool.tile([1, 1], f32)
                nc.vector.tensor_mul(t_aa[:], g1[:], f_s[:, 0:1])
                t_bb = wpool.tile([1, 1], f32)
                nc.vector.tensor_mul(t_bb[:], g2[:], f_s[:, 1:2])
                num = wpool.tile([1, 1], f32)
                nc.vector.scalar_tensor_tensor(
                    num[:], f_s[:, 2:3], 0.5, t_ab[:], Alu.mult, Alu.subtract)
                dt = wpool.tile([1, 1], f32)
                nc.vector.scalar_tensor_tensor(
                    dt[:], f_s[:, 3:4], 0.5, t_aa[:], Alu.mult, Alu.subtract)
                dp = wpool.tile([1, 1], f32)
                nc.vector.scalar_tensor_tensor(
                    dp[:], f_s[:, 4:5], 0.5, t_bb[:], Alu.mult, Alu.subtract)
                den = wpool.tile([1, 1], f32)
                nc.vector.scalar_tensor_tensor(
                    den[:], dt[:], 1.0, dp[:], Alu.mult, Alu.mult)
                dene = wpool.tile([1, 1], f32)
                nc.vector.tensor_scalar(dene[:], den[:], EPS, None, Alu.add)
                sq = wpool.tile([1, 1], f32)
                nc.scalar.activation(sq[:], dene[:], Act.Sqrt)
                rsq = wpool.tile([1, 1], f32)
                nc.vector.reciprocal(rsq[:], sq[:])
                corr = wpool.tile([1, 1], f32)
                nc.vector.tensor_mul(corr[:], num[:], rsq[:])
                loss_t = wpool.tile([1, 1], f32)
                nc.vector.tensor_scalar(loss_t[:], corr[:], -1.0, 1.0,
                                        Alu.mult, Alu.add)
                return loss_t

            prev = tail(gather_summed())
            for _ in range(cc_tail_reps - 1):
                # serial chain: poke the previous loss into the partials so
                # the next AllGather+tail cannot start before it (bench only)
                nc.vector.tensor_copy(part_t[0:1, 0:1], prev[:])
                prev = tail(gather_summed())
            nc.sync.dma_start(loss_d[:], prev[:])

    nc.compile()
    return nc


def _host_tgtT(tgt_loc):
    """Per-core target layout matching the device DMA: chunk (t, k) holds
    tokens {t*TILE_CHUNKS*128 + p*TILE_CHUNKS + k}, so
    tgtT[p, t*TILE_CHUNKS + k] = tgt[t*TC*128 + p*TILE_CHUNKS + k]."""
    return np.ascontiguousarray(
        tgt_loc.reshape(N_TILES, 128, TILE_CHUNKS)
        .transpose(1, 0, 2).reshape(128, N_CHUNKS))


def _host_constants(fine2mid, fine2coarse):
    import ml_dtypes

    f2m = np.asarray(fine2mid, dtype=np.int64)
    f2c = np.asarray(fine2coarse, dtype=np.int64)
    iota = np.ascontiguousarray(np.broadcast_to(
        np.arange(CHUNK, dtype=np.float32),
        (128, OH_BATCH, CHUNK))).reshape(
            128, OH_BATCH * CHUNK).astype(ml_dtypes.bfloat16)
    ident = np.eye(128, dtype=np.float32)
    ones = np.ones((128, 128), dtype=np.float32)
    # selector / averaging matrices from the actual hierarchy inputs
    emt = (f2m[None, :] == np.arange(C_MID)[:, None]).astype(np.float32)
    cnt_m = np.maximum(np.bincount(f2m, minlength=C_MID), 1).astype(np.float32)
    wm = (emt / cnt_m[:, None]).T.astype(np.float32)     # [C_FINE, C_MID]
    # mid2coarse[m] = segment_max of fine2coarse over fines with fine2mid==m
    m2c = np.full(C_MID, -(2**31), dtype=np.int64)
    np.maximum.at(m2c, f2m, f2c)
    emc = (m2c[None, :] == np.arange(C_COARSE)[:, None]).astype(np.float32)
    cnt_c = np.maximum(emc.sum(axis=1), 1).astype(np.float32)
    wc = (emc / cnt_c[:, None]).T.astype(np.float32)     # [C_MID, C_COARSE]
    ect_sel = (f2c[None, :] == np.arange(C_COARSE)[:, None]).astype(np.float32)
    return {
        "iota": iota, "ident": ident, "ones": ones,
        "emt": np.ascontiguousarray(emt),
        "ect": np.ascontiguousarray(ect_sel),
        "wm": np.ascontiguousarray(wm),
        "wc": np.ascontiguousarray(wc),
    }


def _make_in_maps(representations, target_fine, fine2mid, fine2coarse):
    reps = np.ascontiguousarray(np.asarray(representations, dtype=np.float32))
    tgt = np.asarray(target_fine, dtype=np.int32)
    consts = _host_constants(fine2mid, fine2coarse)
    in_maps = []
    for r in range(N_CORES):
        lo, hi = r * N_LOC, (r + 1) * N_LOC
        in_maps.append({
            "reps": reps[lo:hi],
            "tgtT": _host_tgtT(tgt[lo:hi]),
            **consts,
        })
    return in_maps


def kernel(representations, target_fine, fine2mid, fine2coarse):
    from concourse.bass_utils import run_bass_kernel_spmd

    assert np.asarray(representations).shape == (N, D)
    assert np.asarray(target_fine).shape == (N,)

    if "nc" not in _CACHE:
        _CACHE["nc"] = _build_program()
    nc = _CACHE["nc"]

    in_maps = _make_in_maps(representations, target_fine,
                            fine2mid, fine2coarse)
    res = run_bass_kernel_spmd(nc, in_maps, core_ids=list(range(N_CORES)))
    loss = res.results[0]["loss"][0, 0]
    return np.asarray(loss, dtype=np.float32).reshape(())
